# revision 13
# baseline (speedup 1.0000x reference)
"""Trainium2 Bass kernel for per-edge dot products (DGL u_dot_v / DotPredictor).

score[e] = sum_d h[src[e], d] * h[dst[e], d]

Strategy (v3 — fp16 dma_gather with equal-dst descriptor pairing):
  - Split the E=6.4M edges evenly across 8 NeuronCores (800k each); replicate
    the fp16 node table (100000x128, 25.6MB) in each core's HBM.
  - Bulk row gather uses the GPSIMD ucode `dma_gather` (InstDMAGatherAnt).
    The SWDGE rings process ~8ns per descriptor nearly independent of
    descriptor size, so the kernel minimizes DESCRIPTOR COUNT:
      * indices are int16, so the table is viewed as 4 segments of 25000
        rows; edges are bucketed into 16 (src_seg, dst_seg) buckets;
      * within a bucket, edges sharing the same dst node are PAIRED: one
        dst descriptor serves two edges (the multiply re-reads the dst row
        through a stride-0 broadcast AP). ~75% of edges pair, cutting
        descriptors/edge from 2.0 to ~1.6.
  - Bucket chunk lists are sized from the actual data (the program is built
    after seeing the inputs and cached by geometry), with tail chunks rounded
    to 128 indices, so padding is negligible. The SPMD program is shared by
    all 8 cores (per-bucket sizes are maxima over cores).
  - Per chunk: gather rows to SBUF, multiply + per-row reduce on the vector
    engine (fp16 products, fp32 scores), stream scores back to HBM.
  - Host unpermutes scores back to the original edge order.
"""
import sys

sys.path.insert(0, "/opt/trn_rl_repo")

import numpy as np

import concourse.bacc as bacc
import concourse.bass as bass
import concourse.mybir as mybir
import concourse.tile as tile
from concourse.bass_utils import run_bass_kernel_spmd

# Problem shape (hardcoded per contract).
N, E, D = 100000, 6400000, 128
M = 8                      # NeuronCores
P = 128                    # SBUF partitions
E_PER = E // M             # 800000 edges per core
NSEG = 4                   # node-table segments (int16 index range)
S = N // NSEG              # 25000 rows per segment
NBUCKET = NSEG * NSEG      # 16 (src_seg, dst_seg) buckets
GP = 2560                  # pairs per full pair-chunk (dst idx count)
GS = 5120                  # singles per full single-chunk
WP = 2 * (GP // P)         # score cols per pair chunk (40)
WS = GS // P               # score cols per single chunk (40)
SCRATCH = 65536            # SWDGE descriptor-ring carveout bytes
NQUEUES = 4                # SWDGE queues (ucode max)
F16 = mybir.dt.float16
F32 = mybir.dt.float32
I16 = mybir.dt.int16


def _chunk_sizes(count, full):
    """Split count into full-size chunks plus a 128-rounded tail."""
    sizes = [full] * (count // full)
    rem = count % full
    if rem:
        sizes.append(((rem + 127) // 128) * 128)
    if not sizes:
        sizes.append(128)
    return sizes


def build_nc(geom):
    """geom: (pair_sizes, single_sizes) — per bucket, list of chunk sizes."""
    pair_sizes, single_sizes = geom
    nc = bacc.Bacc(
        "TRN2",
        target_bir_lowering=False,
        debug=False,
        dynamic_dma_scratch_size=SCRATCH,
        num_swdge_queues=NQUEUES,
    )
    h = nc.dram_tensor("h", [N, D], F16, kind="ExternalInput")
    IWMAX = GS // 16
    ncp = sum(len(v) for v in pair_sizes)
    ncs = sum(len(v) for v in single_sizes)
    pdix = nc.dram_tensor("pdix", [ncp, P, GP // 16], I16, kind="ExternalInput")
    psix = nc.dram_tensor("psix", [ncp, P, IWMAX], I16, kind="ExternalInput")
    sdix = nc.dram_tensor("sdix", [ncs, P, IWMAX], I16, kind="ExternalInput")
    ssix = nc.dram_tensor("ssix", [ncs, P, IWMAX], I16, kind="ExternalInput")
    outp = nc.dram_tensor("outp", [ncp, P, WP], F32, kind="ExternalOutput")
    outs = nc.dram_tensor("outs", [ncs, P, WS], F32, kind="ExternalOutput")

    q = 0  # SWDGE queue round-robin

    with tile.TileContext(nc) as tc:
        with (
            tc.tile_pool(name="idx", bufs=6) as idx_pool,
            tc.tile_pool(name="rows", bufs=4) as row_pool,
            tc.tile_pool(name="score", bufs=4) as score_pool,
        ):
            cp = 0
            cs = 0
            for k in range(NBUCKET):
                a, b = k // NSEG, k % NSEG
                hs = h[a * S : (a + 1) * S, :]
                hd = h[b * S : (b + 1) * S, :]
                for g in pair_sizes[k]:
                    cols = g // 128
                    idx_d = idx_pool.tile([P, GP // 16], I16, tag="pd")
                    nc.sync.dma_start(
                        out=idx_d[:, : g // 16], in_=pdix[cp, :, : g // 16]
                    )
                    idx_s = idx_pool.tile([P, IWMAX], I16, tag="ps")
                    nc.sync.dma_start(
                        out=idx_s[:, : 2 * g // 16], in_=psix[cp, :, : 2 * g // 16]
                    )
                    d_rows = row_pool.tile([P, (GP // P) * D], F16, tag="pd")
                    s_rows = row_pool.tile([P, 2 * (GP // P) * D], F16, tag="ps")
                    nc.gpsimd.dma_gather(
                        d_rows[:, : cols * D].rearrange("p (c d) -> p c d", d=D),
                        hd, idx_d[:, : g // 16], g, g, D,
                        single_packet=False, queue_num=q % NQUEUES,
                    )
                    q += 1
                    nc.gpsimd.dma_gather(
                        s_rows[:, : 2 * cols * D].rearrange("p (c d) -> p c d", d=D),
                        hs, idx_s[:, : 2 * g // 16], 2 * g, 2 * g, D,
                        single_packet=False, queue_num=q % NQUEUES,
                    )
                    q += 1
                    # prod[p, c, j, :] = s_rows[p, c, j, :] * d_rows[p, c, :]
                    nc.vector.tensor_tensor(
                        out=s_rows[:, : 2 * cols * D].rearrange(
                            "p (c j d) -> p c j d", j=2, d=D
                        ),
                        in0=s_rows[:, : 2 * cols * D].rearrange(
                            "p (c j d) -> p c j d", j=2, d=D
                        ),
                        in1=d_rows[:, : cols * D]
                        .rearrange("p (c d) -> p c d", d=D)[:, :, None, :]
                        .broadcast_to([P, cols, 2, D]),
                        op=mybir.AluOpType.mult,
                    )
                    score = score_pool.tile([P, WP], F32, tag="p")
                    nc.vector.tensor_reduce(
                        out=score[:, : 2 * cols],
                        in_=s_rows[:, : 2 * cols * D].rearrange(
                            "p (c d) -> p c d", d=D
                        ),
                        axis=mybir.AxisListType.X,
                        op=mybir.AluOpType.add,
                    )
                    nc.sync.dma_start(
                        out=outp[cp, :, : 2 * cols], in_=score[:, : 2 * cols]
                    )
                    cp += 1
                for g in single_sizes[k]:
                    cols = g // 128
                    idx_s = idx_pool.tile([P, IWMAX], I16, tag="ss")
                    nc.sync.dma_start(
                        out=idx_s[:, : g // 16], in_=ssix[cs, :, : g // 16]
                    )
                    idx_d = idx_pool.tile([P, IWMAX], I16, tag="sd")
                    nc.sync.dma_start(
                        out=idx_d[:, : g // 16], in_=sdix[cs, :, : g // 16]
                    )
                    s_rows = row_pool.tile([P, WS * D], F16, tag="ss")
                    d_rows = row_pool.tile([P, WS * D], F16, tag="sd")
                    nc.gpsimd.dma_gather(
                        s_rows[:, : cols * D].rearrange("p (c d) -> p c d", d=D),
                        hs, idx_s[:, : g // 16], g, g, D,
                        single_packet=False, queue_num=q % NQUEUES,
                    )
                    q += 1
                    nc.gpsimd.dma_gather(
                        d_rows[:, : cols * D].rearrange("p (c d) -> p c d", d=D),
                        hd, idx_d[:, : g // 16], g, g, D,
                        single_packet=False, queue_num=q % NQUEUES,
                    )
                    q += 1
                    nc.vector.tensor_tensor(
                        out=s_rows[:, : cols * D],
                        in0=s_rows[:, : cols * D],
                        in1=d_rows[:, : cols * D],
                        op=mybir.AluOpType.mult,
                    )
                    score = score_pool.tile([P, WS], F32, tag="s")
                    nc.vector.tensor_reduce(
                        out=score[:, : cols],
                        in_=s_rows[:, : cols * D].rearrange(
                            "p (c d) -> p c d", d=D
                        ),
                        axis=mybir.AxisListType.X,
                        op=mybir.AluOpType.add,
                    )
                    nc.sync.dma_start(
                        out=outs[cs, :, : cols], in_=score[:, : cols]
                    )
                    cs += 1
    nc.compile()
    return nc


_NC_CACHE = {}


def _get_nc(geom_key, geom):
    nc = _NC_CACHE.get(geom_key)
    if nc is None:
        nc = build_nc(geom)
        _NC_CACHE[geom_key] = nc
    return nc


def _wrap(arr2d):
    """[nchunks, idx] -> wrapped+tiled [nchunks, P, idx/16] int16."""
    nch, g = arr2d.shape
    a = arr2d.reshape(nch, g // 16, 16).transpose(0, 2, 1)
    return np.ascontiguousarray(np.tile(a, (1, P // 16, 1)))


def _seq_within(groups, ngroups):
    """Sequential index of each element within its group (groups sorted)."""
    cnt = np.bincount(groups, minlength=ngroups)
    starts = np.r_[0, np.cumsum(cnt)[:-1]]
    return np.arange(len(groups), dtype=np.int64) - starts[groups]


def _prep_core(src_c, dst_c):
    """Equal-dst pair/single decomposition for one core."""
    b = (src_c // S) * NSEG + (dst_c // S)
    key = b * N + dst_c                      # sort by (bucket, dst node)
    order = np.argsort(key, kind="stable")
    ks = key[order]
    run_start = np.r_[0, np.flatnonzero(ks[1:] != ks[:-1]) + 1]
    run_len_r = np.diff(np.r_[run_start, len(ks)])
    run_len = np.repeat(run_len_r, run_len_r)
    rank = np.arange(E_PER, dtype=np.int64) - np.repeat(run_start, run_len_r)
    paired = (rank | 1) < run_len
    even = paired & (rank % 2 == 0)

    bs = b[order]
    pair_cnt = np.bincount(bs[even], minlength=NBUCKET)
    sing_cnt = np.bincount(bs[~paired], minlength=NBUCKET)

    lead_idx = np.flatnonzero(even)
    sing_idx = np.flatnonzero(~paired)
    return dict(
        pair_cnt=pair_cnt, sing_cnt=sing_cnt,
        lead_b=bs[lead_idx], pair_seq=_seq_within(bs[lead_idx], NBUCKET),
        e_lead=order[lead_idx], e_part=order[lead_idx + 1],
        sing_b=bs[sing_idx], sing_seq=_seq_within(bs[sing_idx], NBUCKET),
        e_sing=order[sing_idx],
        src=src_c, dst=dst_c,
    )


def _geometry(preps):
    pair_max = np.max([p["pair_cnt"] for p in preps], axis=0)
    sing_max = np.max([p["sing_cnt"] for p in preps], axis=0)
    pair_sizes = [_chunk_sizes(int(c), GP) for c in pair_max]
    single_sizes = [_chunk_sizes(int(c), GS) for c in sing_max]
    return pair_sizes, single_sizes


def _chunk_layout(sizes_per_bucket):
    """Per bucket: (first_chunk_id, sizes array, exclusive-prefix offsets)."""
    layout = []
    c0 = 0
    for sizes in sizes_per_bucket:
        arr = np.asarray(sizes, dtype=np.int64)
        layout.append((c0, arr, np.r_[0, np.cumsum(arr)[:-1]]))
        c0 += len(sizes)
    return layout, c0


def _place(groups, seqs, layout):
    """Map (bucket, within-bucket seq) -> (chunk id, slot in chunk)."""
    chunk_of = np.empty(len(groups), dtype=np.int64)
    slot_of = np.empty(len(groups), dtype=np.int64)
    for k in range(NBUCKET):
        m = groups == k
        c0, sizes, offs = layout[k]
        sq = seqs[m]
        ci = np.searchsorted(np.cumsum(sizes), sq, side="right")
        chunk_of[m] = c0 + ci
        slot_of[m] = sq - offs[ci]
    return chunk_of, slot_of


def _build_core_arrays(prep, geom):
    pair_sizes, single_sizes = geom
    p_layout, ncp = _chunk_layout(pair_sizes)
    s_layout, ncs = _chunk_layout(single_sizes)

    pdix = np.zeros((ncp, GP), dtype=np.int16)
    psix = np.zeros((ncp, 2 * GP), dtype=np.int16)
    sdix = np.zeros((ncs, GS), dtype=np.int16)
    ssix = np.zeros((ncs, GS), dtype=np.int16)

    # pairs: dst idx at slot t; src idx of edge j at linear position
    # (2*(t//P) + j)*P + t%P  (dma_gather sends idx i to partition i%P,
    # column i//P).
    chunk_of, slot_of = _place(prep["lead_b"], prep["pair_seq"], p_layout)
    pdix[chunk_of, slot_of] = (prep["dst"][prep["e_lead"]] % S).astype(np.int16)
    pr, pc = slot_of % P, slot_of // P
    psix[chunk_of, (2 * pc) * P + pr] = (prep["src"][prep["e_lead"]] % S).astype(np.int16)
    psix[chunk_of, (2 * pc + 1) * P + pr] = (prep["src"][prep["e_part"]] % S).astype(np.int16)
    pos_lead = (chunk_of * P + pr) * WP + 2 * pc
    pos_part = pos_lead + 1

    # singles
    chunk_of, slot_of = _place(prep["sing_b"], prep["sing_seq"], s_layout)
    sdix[chunk_of, slot_of] = (prep["dst"][prep["e_sing"]] % S).astype(np.int16)
    ssix[chunk_of, slot_of] = (prep["src"][prep["e_sing"]] % S).astype(np.int16)
    pos_sing = (chunk_of * P + slot_of % P) * WS + slot_of // P

    in_map = {
        "pdix": _wrap(pdix),
        "psix": _wrap(psix),
        "sdix": _wrap(sdix),
        "ssix": _wrap(ssix),
    }
    meta = (
        np.concatenate([prep["e_lead"], prep["e_part"]]),
        np.concatenate([pos_lead, pos_part]),
        prep["e_sing"],
        pos_sing,
    )
    return in_map, meta


def build_in_maps(inputs):
    """Host-side staging. Returns (geom_key, geom, in_maps, metas)."""
    h = np.ascontiguousarray(np.asarray(inputs["h"], dtype=np.float32))
    src = np.ascontiguousarray(np.asarray(inputs["src"]).astype(np.int64))
    dst = np.ascontiguousarray(np.asarray(inputs["dst"]).astype(np.int64))
    assert h.shape == (N, D) and src.shape == (E,) and dst.shape == (E,)
    h16 = h.astype(np.float16)

    preps = []
    for i in range(M):
        sl = slice(i * E_PER, (i + 1) * E_PER)
        preps.append(_prep_core(src[sl], dst[sl]))
    geom = _geometry(preps)
    geom_key = tuple(tuple(tuple(s) for s in side) for side in geom)

    in_maps, metas = [], []
    for i in range(M):
        in_map, meta = _build_core_arrays(preps[i], geom)
        in_map["h"] = h16
        in_maps.append(in_map)
        metas.append(meta)
    return geom_key, geom, in_maps, metas


def run(inputs, trace=False, trace_kwargs=None):
    """Shard, execute on 8 cores, gather. Returns (scores[E] fp32, results)."""
    geom_key, geom, in_maps, metas = build_in_maps(inputs)
    nc = _get_nc(geom_key, geom)

    try:
        res = run_bass_kernel_spmd(
            nc, in_maps, core_ids=list(range(M)),
            trace=trace, trace_kwargs=trace_kwargs or {},
        )
    except ModuleNotFoundError:
        # axon build without NTFF profiling hooks — run without trace
        res = run_bass_kernel_spmd(nc, in_maps, core_ids=list(range(M)), trace=False)

    scores = np.empty(E, dtype=np.float32)
    for i in range(M):
        sl = slice(i * E_PER, (i + 1) * E_PER)
        e_pair, pos_pair, e_sing, pos_sing = metas[i]
        flat_p = np.asarray(res.results[i]["outp"], dtype=np.float32).reshape(-1)
        flat_s = np.asarray(res.results[i]["outs"], dtype=np.float32).reshape(-1)
        sc = np.empty(E_PER, dtype=np.float32)
        sc[e_pair] = flat_p[pos_pair]
        sc[e_sing] = flat_s[pos_sing]
        scores[sl] = sc
    return scores, res


def kernel(**inputs) -> np.ndarray:
    return run(inputs)[0]


# revision 15
# speedup vs baseline: 1.3468x; 1.3468x over previous
"""Trainium2 Bass kernel for per-edge dot products (DGL u_dot_v / DotPredictor).

score[e] = sum_d h[src[e], d] * h[dst[e], d]

Strategy (fp16 table + deep-pipelined SWDGE gathers):
  - Split the E=6.4M edges evenly across 8 NeuronCores (800k each); replicate
    the node table h as fp16 (100000x128, 25.6MB) in each core's HBM.
  - Bulk row gather uses the GPSIMD ucode `dma_gather` (InstDMAGatherAnt):
    5120 256B row fetches per instruction across the 4 SWDGE queues. Indices
    are int16, so the node table is viewed as 4 segments of 25000 rows and
    each core's edges are bucketed on the host into 16 (src_seg, dst_seg)
    buckets (padded to a fixed size so the SPMD program is static). Edges
    past the pad (won't happen for the expected distribution) fall back to a
    host dot product.
  - Deep tile pools (idx 8, rows 7) keep many gathers in flight; with only
    2 buffers the idx-slot dependency caps gather concurrency at 2 and the
    kernel runs >20x slower.
  - Per 5120-edge chunk: gather h[src] and h[dst] rows to SBUF, multiply +
    per-row reduce on the vector engine (fp16 in, fp32 scores), stream 1
    score/edge back to HBM. Host unpermutes to original edge order.
"""
import sys

sys.path.insert(0, "/opt/trn_rl_repo")

import numpy as np

import concourse.bacc as bacc
import concourse.bass as bass
import concourse.mybir as mybir
import concourse.tile as tile
from concourse.bass_utils import run_bass_kernel_spmd

# Problem shape (hardcoded per contract).
N, E, D = 100000, 6400000, 128
M = 8                      # NeuronCores
P = 128                    # SBUF partitions
E_PER = E // M             # 800000 edges per core
NSEG = 4                   # node-table segments (int16 index range)
S = N // NSEG              # 25000 rows per segment
NBUCKET = NSEG * NSEG      # 16 (src_seg, dst_seg) buckets
G = 5120                   # indices per dma_gather
CPG = G // P               # 50 dst columns per gather
B_PAD = 51200              # padded bucket size (10 chunks of G)
NCPB = B_PAD // G          # 8 chunks per bucket
TC = NBUCKET * NCPB        # 128 chunks per core
IW = G // 16               # idx columns per chunk (wrapped layout)
SCRATCH = 65536            # SWDGE descriptor-ring carveout bytes
SINGLE_PACKET = False      # one giant packet overflows the SWDGE ring; use
                           # multi-packet mode so the ucode reclaims space
NQUEUES = 4                # SWDGE queues: parallel Q7 descriptor generation


def build_nc():
    nc = bacc.Bacc(
        "TRN2",
        target_bir_lowering=False,
        debug=False,
        dynamic_dma_scratch_size=SCRATCH,
        num_swdge_queues=NQUEUES,
    )
    h = nc.dram_tensor("h", [N, D], mybir.dt.float16, kind="ExternalInput")
    sidx = nc.dram_tensor("sidx", [TC, P, IW], mybir.dt.int16, kind="ExternalInput")
    didx = nc.dram_tensor("didx", [TC, P, IW], mybir.dt.int16, kind="ExternalInput")
    out = nc.dram_tensor("out", [TC, P, CPG], mybir.dt.float32, kind="ExternalOutput")

    with tile.TileContext(nc) as tc:
        with (
            tc.tile_pool(name="idx", bufs=8) as idx_pool,
            tc.tile_pool(name="rows", bufs=7) as row_pool,
            tc.tile_pool(name="score", bufs=4) as score_pool,
        ):
            for c in range(TC):
                k = c // NCPB
                a, b = k // NSEG, k % NSEG
                idx_s = idx_pool.tile([P, IW], mybir.dt.int16, tag="s")
                idx_d = idx_pool.tile([P, IW], mybir.dt.int16, tag="d")
                nc.sync.dma_start(out=idx_s[:], in_=sidx[c])
                nc.sync.dma_start(out=idx_d[:], in_=didx[c])
                s_rows = row_pool.tile([P, CPG * D], mybir.dt.float16, tag="s")
                d_rows = row_pool.tile([P, CPG * D], mybir.dt.float16, tag="d")
                nc.gpsimd.dma_gather(
                    s_rows[:].rearrange("p (c d) -> p c d", d=D),
                    h[a * S : (a + 1) * S, :],
                    idx_s[:],
                    G,
                    G,
                    D,
                    single_packet=SINGLE_PACKET,
                    queue_num=(2 * c) % NQUEUES,
                )
                nc.gpsimd.dma_gather(
                    d_rows[:].rearrange("p (c d) -> p c d", d=D),
                    h[b * S : (b + 1) * S, :],
                    idx_d[:],
                    G,
                    G,
                    D,
                    single_packet=SINGLE_PACKET,
                    queue_num=(2 * c + 1) % NQUEUES,
                )
                nc.vector.tensor_tensor(
                    out=s_rows[:],
                    in0=s_rows[:],
                    in1=d_rows[:],
                    op=mybir.AluOpType.mult,
                )
                score = score_pool.tile([P, CPG], mybir.dt.float32, tag="sc")
                nc.vector.tensor_reduce(
                    out=score[:],
                    in_=s_rows[:].rearrange("p (c d) -> p c d", d=D),
                    axis=mybir.AxisListType.X,
                    op=mybir.AluOpType.add,
                )
                nc.sync.dma_start(out=out[c], in_=score[:])
    nc.compile()
    return nc


_NC_CACHE = None


def _get_nc():
    global _NC_CACHE
    if _NC_CACHE is None:
        _NC_CACHE = build_nc()
    return _NC_CACHE


def _prep_core(src_c, dst_c):
    """Bucket one core's edges. Returns (sidx, didx, pos, keep) where
    sidx/didx are the wrapped [TC, P, IW] int16 device index tensors, pos is
    each kept edge's flat position in the bucketed stream, keep the mask."""
    b = (src_c // S).astype(np.int32) * NSEG + (dst_c // S).astype(np.int32)
    # rank of each edge within its bucket, in original order
    rank = np.empty(E_PER, dtype=np.int64)
    for k in range(NBUCKET):
        m = b == k
        rank[m] = np.arange(m.sum(), dtype=np.int64)
    keep = rank < B_PAD
    pos = b.astype(np.int64) * B_PAD + rank  # valid where keep

    spad = np.zeros(NBUCKET * B_PAD, dtype=np.int16)
    dpad = np.zeros(NBUCKET * B_PAD, dtype=np.int16)
    kp = pos[keep]
    spad[kp] = (src_c[keep] % S).astype(np.int16)
    dpad[kp] = (dst_c[keep] % S).astype(np.int16)

    def wrap(arr):
        # [NBUCKET*B_PAD] -> [TC, G] -> wrapped [TC, 16, IW] -> tiled [TC, P, IW]
        a = arr.reshape(TC, IW, 16).transpose(0, 2, 1)
        return np.ascontiguousarray(np.tile(a, (1, P // 16, 1)))

    return wrap(spad), wrap(dpad), pos, keep


def build_in_maps(inputs):
    """Host-side staging: shard edges, bucket, wrap indices.
    Returns (in_maps, metas)."""
    h = np.ascontiguousarray(np.asarray(inputs["h"], dtype=np.float32))
    src = np.ascontiguousarray(np.asarray(inputs["src"]).astype(np.int32))
    dst = np.ascontiguousarray(np.asarray(inputs["dst"]).astype(np.int32))
    assert h.shape == (N, D) and src.shape == (E,) and dst.shape == (E,)
    h16 = h.astype(np.float16)

    in_maps = []
    metas = []
    for i in range(M):
        sl = slice(i * E_PER, (i + 1) * E_PER)
        sidx, didx, pos, keep = _prep_core(src[sl], dst[sl])
        in_maps.append({"h": h16, "sidx": sidx, "didx": didx})
        metas.append((pos, keep))
    return in_maps, metas


def run(inputs, trace=False, trace_kwargs=None):
    """Shard, execute on 8 cores, gather. Returns (scores[E] fp32, results)."""
    h = np.asarray(inputs["h"], dtype=np.float32)
    src = np.asarray(inputs["src"]).astype(np.int32)
    dst = np.asarray(inputs["dst"]).astype(np.int32)
    in_maps, metas = build_in_maps(inputs)

    try:
        res = run_bass_kernel_spmd(
            _get_nc(),
            in_maps,
            core_ids=list(range(M)),
            trace=trace,
            trace_kwargs=trace_kwargs or {},
        )
    except ModuleNotFoundError:
        # axon build without NTFF profiling hooks — run without trace
        res = run_bass_kernel_spmd(
            _get_nc(), in_maps, core_ids=list(range(M)), trace=False
        )

    scores = np.empty(E, dtype=np.float32)
    for i in range(M):
        sl = slice(i * E_PER, (i + 1) * E_PER)
        pos, keep = metas[i]
        out_arr = np.asarray(res.results[i]["out"], dtype=np.float32)
        # out_arr[c, p, j] is the score of bucketed position c*G + j*128 + p
        flat = out_arr.transpose(0, 2, 1).reshape(-1)
        sc = np.empty(E_PER, dtype=np.float32)
        sc[keep] = flat[pos[keep]]
        if not keep.all():  # host fallback for bucket-overflow edges
            ov = ~keep
            sc[ov] = np.einsum(
                "ed,ed->e", h[src[sl][ov]], h[dst[sl][ov]]
            ).astype(np.float32)
        scores[sl] = sc
    return scores, res


def kernel(**inputs) -> np.ndarray:
    return run(inputs)[0]

